# revision 34
# baseline (speedup 1.0000x reference)
"""Bahdanau additive attention on 8 Trainium2 NeuronCores.

Data-parallel over batch: core c handles batches [4c, 4c+4).
Per batch b:
  ep[k,t]   = sum_h Ua[k,h] * enc[b,t,h]        (bf16 PE matmuls, k on PSUM partitions)
  z[k,t]    = tanh(ep[k,t] + hp[b,k])           (ScalarE, hp as per-partition bias)
  e[t]      = sum_k va[k] * z[k,t]              (M=1 bf16 PE matmuls, 4x col-packed)
  attn      = softmax(e) * mask renorm          (strip-exp out of PSUM; no max-sub)
  ctx[h]    = sum_t attn[t] * enc[b,t,h]        (M=1 bf16 PE matmuls, 4x col-packed,
                                                 1/sum folded into the exp-transpose)
hp[b,k] = sum_h Wa[k,h] * h_t[b,h] runs as per-kt N=4 matmul chains whose tiny
weight chunks trickle in with the staged weight DMAs, filling the DMA-starved
prologue. All one-time weights (Ua blocks, Wa blocks, va, h_t^T) are packed into
a single [128, WX] DRAM tensor loaded with a handful of staged DMAs (descriptor
issue on the Sync engine costs ~0.7us per dma_start).
"""

import numpy as np

import concourse.bass as bass
import concourse.tile as tile
from concourse import bacc, mybir

dt = mybir.dt
AF = mybir.ActivationFunctionType

B, T, H = 32, 1024, 1024
NCORES = 8
BL = B // NCORES          # batches per core
P = 128                   # partitions
NT = 512                  # matmul free-dim chunk (one PSUM bank of fp32)
KT = H // P               # k-tiles (output rows of ep)
HT = H // P               # h-tiles (contraction)
TT = T // P               # t-tiles (partition tiles of natural enc)
TC = T // NT              # t chunks per batch
EC = 2                    # e/ctx col-packed chunks
EW = T // EC              # 256

# packed-weight column offsets (bf16 elements per partition)
OFF_UA0 = 0
OFF_VA = 1024
OFF_HTT = 1032
OFF_WA0 = 1064
OFF_REST = 2088           # kt>=1: [uaT_kt (1024) | waT_kt (1024)] blocks
WX = OFF_REST + (KT - 1) * 2048


def _off_ua(kt):
    return OFF_UA0 if kt == 0 else OFF_REST + (kt - 1) * 2048


def _off_wa(kt):
    return OFF_WA0 if kt == 0 else OFF_REST + (kt - 1) * 2048 + 1024


_CACHE = {}


def _build_nc():
    nc = bacc.Bacc("TRN2", target_bir_lowering=False, debug=False)

    encT_d = nc.dram_tensor("encT", [BL, TC, P, HT, NT], dt.bfloat16,
                            kind="ExternalInput").ap()
    encn_d = nc.dram_tensor("encn", [BL, TC, P, TT, NT], dt.bfloat16,
                            kind="ExternalInput").ap()
    wall_d = nc.dram_tensor("wall", [P, WX], dt.bfloat16,
                            kind="ExternalInput").ap()
    mask_d = nc.dram_tensor("mask", [BL, T], dt.uint8, kind="ExternalInput").ap()

    ctx_d = nc.dram_tensor("ctx", [BL, H], dt.float32, kind="ExternalOutput").ap()
    attn_d = nc.dram_tensor("attn", [BL, T], dt.float32, kind="ExternalOutput").ap()

    with tile.TileContext(nc) as tc:
        from contextlib import ExitStack

        with ExitStack() as st:
            wpool = st.enter_context(tc.tile_pool(name="weights", bufs=1))
            etpool = st.enter_context(tc.tile_pool(name="encT", bufs=4))
            natpool = st.enter_context(tc.tile_pool(name="nat", bufs=4))
            thpool = st.enter_context(tc.tile_pool(name="tanh", bufs=4))
            smpool = st.enter_context(tc.tile_pool(name="small", bufs=1))
            pmain = st.enter_context(tc.tile_pool(name="pmain", bufs=4, space="PSUM"))
            pe_ps = st.enter_context(tc.tile_pool(name="pe", bufs=2, space="PSUM"))
            ptail = st.enter_context(tc.tile_pool(name="ptail", bufs=2, space="PSUM"))

            wall_sb = wpool.tile([P, WX], dt.bfloat16, tag="wall")

            def uaT_ap(kt, ht):
                o = _off_ua(kt) + ht * P
                return wall_sb[:, o:o + P]

            def waT_ap(kt, ht):
                o = _off_wa(kt) + ht * P
                return wall_sb[:, o:o + P]

            def htT_ap(ht):
                o = OFF_HTT + ht * BL
                return wall_sb[:, o:o + BL]

            def va_ap(kt):
                return wall_sb[:, OFF_VA + kt:OFF_VA + kt + 1]

            def load_wall(c0, c1):
                nc.sync.dma_start(wall_sb[:, c0:c1], wall_d[:, c0:c1])

            def load_encT(bi, tcc, eng, split=False):
                t_ = etpool.tile([P, HT, NT], dt.bfloat16, tag="encT",
                                 name=f"encT{bi}_{tcc}")
                if split:
                    # alternate halves across the two HWDGE queues so more
                    # DMA engines engage concurrently in the prologue
                    nc.sync.dma_start(t_[:, 0:2, :], encT_d[bi, tcc, :, 0:2, :])
                    nc.scalar.dma_start(t_[:, 2:4, :], encT_d[bi, tcc, :, 2:4, :])
                    nc.sync.dma_start(t_[:, 4:6, :], encT_d[bi, tcc, :, 4:6, :])
                    nc.scalar.dma_start(t_[:, 6:8, :], encT_d[bi, tcc, :, 6:8, :])
                else:
                    eng.dma_start(t_[:], encT_d[bi, tcc])
                return t_

            def load_nat(bi, kc):
                t_ = natpool.tile([P, TT, NT], dt.bfloat16, tag="nat",
                                  name=f"nat{bi}_{kc}")
                nc.scalar.dma_start(t_[:], encn_d[bi, kc])
                return t_

            def load_wall2(c0, c1, eng):
                eng.dma_start(wall_sb[:, c0:c1], wall_d[:, c0:c1])

            # prologue DMAs, need-ordered, striped across both queues:
            # per kt, Ua block rides the sync queue and Wa block the scalar
            # queue so both queues advance one half-block per group.
            load_wall2(OFF_UA0, OFF_UA0 + 512, nc.sync)       # uaT0 a
            load_wall2(OFF_UA0 + 512, OFF_VA, nc.scalar)      # uaT0 b
            load_wall2(OFF_VA, OFF_WA0, nc.sync)              # va + htT (tiny)
            encT_b0 = [load_encT(0, 0, None, split=True)]
            load_wall2(OFF_WA0, OFF_WA0 + 512, nc.sync)       # waT0 a
            load_wall2(OFF_WA0 + 512, OFF_REST, nc.scalar)    # waT0 b
            for _kt in range(1, KT):
                load_wall2(_off_ua(_kt), _off_wa(_kt), nc.sync)    # uaT_kt
                load_wall2(_off_wa(_kt), _off_ua(_kt + 1) if _kt < KT - 1
                           else WX, nc.scalar)                     # waT_kt

            # PE warm-up: back-to-back dummy matmuls while the prologue DMAs
            # stream in, so HAM reaches K=8/8 before the first real group.
            # The memset goes FIRST on the DVE queue so the warm-up isn't
            # delayed behind the 4.3us mask pass below.
            ones_b = wpool.tile([1, 1], dt.bfloat16, tag="ones_b")
            nc.vector.memset(ones_b[:], 1.0)
            hp_sb = wpool.tile([P, KT, BL], dt.float32, tag="hp")
            g_rhs = wpool.tile([P, NT], dt.bfloat16, tag="g_rhs")
            nc.vector.memset(g_rhs[:], 0.0)
            warm_ps = ptail.tile([P, NT], dt.float32, tag="tailps",
                                 name="warm_ps")
            for _ in range(16):
                nc.tensor.matmul(warm_ps[:], g_rhs[:, :P], g_rhs[:],
                                 start=True, stop=True)

            # mask for all batches: cast-DMA + one dual-op DVE pass (DVE is
            # idle in the prologue; ScalarE is not): mask_m1 = (mask-1)*1e30
            mask_f = smpool.tile([1, BL * T], dt.float32, tag="mask")
            nc.gpsimd.dma_start(mask_f[:], mask_d.rearrange("b t -> (b t)"))
            mask_m1 = smpool.tile([1, BL * T], dt.bfloat16, tag="mask_m1")
            nc.vector.tensor_scalar(mask_m1[:], mask_f[:], 1.0, 1e30,
                                    mybir.AluOpType.subtract,
                                    mybir.AluOpType.mult)
            encT_b0.append(load_encT(0, 1, None, split=True))

            # ---- deferred post-op FIFO: one item emitted per main group ----
            post_q = []

            def pop_post():
                if post_q:
                    post_q.pop(0)()

            def make_epack(e_tile, kt, th0, th1):
                def emit():
                    for c in range(EC):
                        pos = c * EW
                        th = th0 if pos < NT else th1
                        nc.tensor.matmul(
                            e_tile[32 * c:32 * c + 1, :EW],
                            va_ap(kt),
                            th[:, pos % NT:pos % NT + EW],
                            start=(kt == 0), stop=False,
                            tile_position=(0, 32 * c))
                return emit

            def make_mask_exp(e_tile, ex_row, ssum4, boxes, bi, inline_atp,
                              nat_kc=None):
                def emit():
                    atp = ptail.tile([P, TT], dt.float32, tag="tailps",
                                     name=f"atp{bi}")
                    boxes["atp"] = atp
                    if inline_atp:
                        boxes["attnT"] = smpool.tile(
                            [P, TT], dt.bfloat16, tag="attnT", bufs=2,
                            name=f"attnT{bi}")
                        boxes["cp"] = ptail.tile([P, NT], dt.float32,
                                                 tag="tailps", name=f"cp{bi}")
                    for c in range(EC):
                        nc.tensor.matmul(
                            e_tile[32 * c:32 * c + 1, :EW],
                            ones_b[:],
                            mask_m1[0:1, bi * T + c * EW:bi * T + (c + 1) * EW],
                            start=False, stop=True,
                            tile_position=(0, 32 * c))
                    # strip-exp straight out of PSUM into a [1, T] row
                    # (partition shift 32c -> 0), then per-chunk sums on DVE
                    # pipelined behind the ScalarE exps. On the last batch the
                    # exps run in 8 half-strips and each half immediately
                    # feeds its exp-transpose + ctx matmuls (ScalarE/PE
                    # pipeline in the exposed tail).
                    if not inline_atp:
                        for c in range(EC):
                            nc.scalar.activation(
                                ex_row[:, c * EW:(c + 1) * EW],
                                e_tile[32 * c:32 * c + 1, :EW], AF.Exp)
                            nc.vector.tensor_reduce(
                                ssum4[:, c:c + 1],
                                ex_row[:, c * EW:(c + 1) * EW],
                                axis=mybir.AxisListType.X,
                                op=mybir.AluOpType.add)
                    else:
                        per = TT // EC
                        for tt in range(TT):
                            c, h2 = tt // per, tt % per
                            nc.scalar.activation(
                                ex_row[:, tt * P:(tt + 1) * P],
                                e_tile[32 * c:32 * c + 1,
                                       h2 * P:(h2 + 1) * P], AF.Exp)
                            nc.vector.tensor_reduce(
                                ssum4[:, tt:tt + 1],
                                ex_row[:, tt * P:(tt + 1) * P],
                                axis=mybir.AxisListType.X,
                                op=mybir.AluOpType.add)
                            nc.tensor.matmul(
                                atp[:, tt:tt + 1],
                                ex_row[:, tt * P:(tt + 1) * P],
                                ones_b[:], start=True, stop=True)
                            nc.vector.tensor_copy(
                                boxes["attnT"][:, tt:tt + 1],
                                atp[:, tt:tt + 1])
                            for cc in range(EC):
                                nc.tensor.matmul(
                                    boxes["cp"][32 * cc:32 * cc + 1, :EW],
                                    boxes["attnT"][:, tt:tt + 1],
                                    nat_kc[(cc * EW) // NT][
                                        :, tt,
                                        (cc * EW) % NT:(cc * EW) % NT + EW],
                                    start=(tt == 0), stop=(tt == TT - 1),
                                    tile_position=(0, 32 * cc))
                return emit

            def make_softmax(ssum4, rinv, ncols):
                def emit():
                    ssum = smpool.tile([1, 1], dt.float32, tag="ssum", bufs=2)
                    nc.vector.tensor_reduce(ssum[:], ssum4[:, 0:ncols],
                                            axis=mybir.AxisListType.X,
                                            op=mybir.AluOpType.add)
                    nc.vector.reciprocal(rinv[:], ssum[:])
                return emit

            def make_tail(bi, ex_row, rinv, boxes, nat_kc, inline_atp):
                def emit():
                    # transpose UNnormalized exp into partitions: 1/sum is
                    # applied later on the ctx strips, so this does not wait
                    # for the softmax sum.
                    atp = boxes["atp"]
                    if not inline_atp:
                        for tt in range(TT):
                            nc.tensor.matmul(
                                atp[:, tt:tt + 1],
                                ex_row[:, tt * P:(tt + 1) * P],
                                ones_b[:], start=True, stop=True)
                        attnT = smpool.tile([P, TT], dt.bfloat16, tag="attnT",
                                            bufs=2)
                        nc.vector.tensor_copy(attnT[:], atp[:])
                    else:
                        attnT = boxes["attnT"]
                    # attn output: ex * (1/sum), full fp32 row
                    attn_sb = smpool.tile([1, T], dt.float32, tag="attn",
                                          bufs=2)
                    nc.vector.tensor_scalar_mul(attn_sb[:], ex_row[:], rinv[:])
                    nc.scalar.dma_start(attn_d[bi:bi + 1, :], attn_sb[:])
                    # context: normalize while draining the PSUM strips
                    if not inline_atp:
                        cp = ptail.tile([P, NT], dt.float32, tag="tailps",
                                        name=f"cp{bi}")
                        for tt in range(TT):
                            for c in range(EC):
                                nc.tensor.matmul(
                                    cp[32 * c:32 * c + 1, :EW],
                                    attnT[:, tt:tt + 1],
                                    nat_kc[(c * EW) // NT][
                                        :, tt,
                                        (c * EW) % NT:(c * EW) % NT + EW],
                                    start=(tt == 0), stop=(tt == TT - 1),
                                    tile_position=(0, 32 * c))
                    else:
                        cp = boxes["cp"]
                    ctx_sb = smpool.tile([1, H], dt.float32, tag="ctx", bufs=2)
                    for c in range(EC):
                        if c % 2 == 0:
                            nc.vector.tensor_scalar_mul(
                                ctx_sb[:, c * EW:(c + 1) * EW],
                                cp[32 * c:32 * c + 1, :EW], rinv[:])
                        else:
                            nc.scalar.mul(
                                ctx_sb[:, c * EW:(c + 1) * EW],
                                cp[32 * c:32 * c + 1, :EW], rinv[:])
                    nc.scalar.dma_start(ctx_d[bi:bi + 1, :], ctx_sb[:])
                return emit

            # ---- main loop: tcc-outer for batch 0 (DMA need-order),
            # kt-outer for the rest (both encT tiles prefetched) ----
            for bi in range(BL):
                if bi == 0:
                    encT_t = encT_b0
                    group_iter = [(kt, tcc) for tcc in range(TC)
                                  for kt in range(KT)]
                elif bi == 1:
                    encT_t = encT_b1
                else:
                    encT_t = encT_next
                if bi > 0:
                    group_iter = [(kt, tcc) for kt in range(KT)
                                  for tcc in range(TC)]
                e_tile = pe_ps.tile([P, NT], dt.float32, tag="e",
                                    name=f"e_ps{bi}")
                ex_row = smpool.tile([1, T], dt.bfloat16, tag="ex", bufs=2,
                                     name=f"ex{bi}")
                ssum4 = smpool.tile([1, TT], dt.float32, tag="ssum4", bufs=2,
                                    name=f"ssum4_{bi}")
                rinv = smpool.tile([1, 1], dt.float32, tag="rinv", bufs=2,
                                   name=f"rinv{bi}")
                boxes = {}
                th0_of = {}
                for gi, (kt, tcc) in enumerate(group_iter):
                    # prefetch emission points
                    if bi == 0:
                        if gi == 10:
                            encT_b1 = [load_encT(1, 0, nc.sync)]
                            nat_kc = [load_nat(bi, 0)]
                        elif gi == 12:
                            encT_b1.append(load_encT(1, 1, nc.sync))
                            nat_kc.append(load_nat(bi, 1))
                    else:
                        if gi == 2 and bi < BL - 1:
                            encT_next = [load_encT(bi + 1, 0, nc.sync)]
                        elif gi == 6 and bi < BL - 1:
                            encT_next.append(load_encT(bi + 1, 1, nc.sync))
                        if gi == 10:
                            nat_kc = [load_nat(bi, 0)]
                        elif gi == 12:
                            nat_kc.append(load_nat(bi, 1))
                    ps = pmain.tile([P, NT], dt.float32, tag="big")
                    for ht in range(HT):
                        nc.tensor.matmul(
                            ps[:], uaT_ap(kt, ht), encT_t[tcc][:, ht, :],
                            start=(ht == 0), stop=(ht == HT - 1))
                    if bi == 0 and tcc == 0:
                        hp_ps = ptail.tile([P, BL], dt.float32, tag="tailps",
                                           name=f"hp_ps{kt}")
                        for ht in range(HT):
                            nc.tensor.matmul(
                                hp_ps[:], waT_ap(kt, ht), htT_ap(ht),
                                start=(ht == 0), stop=(ht == HT - 1))
                        nc.vector.tensor_copy(hp_sb[:, kt, :], hp_ps[:])
                    th = thpool.tile([P, NT], dt.bfloat16, tag="th",
                                     bufs=12, name="th")
                    nc.scalar.activation(th[:], ps[:], AF.Tanh,
                                         bias=hp_sb[:, kt, bi:bi + 1])
                    pop_post()
                    if tcc == 0:
                        th0_of[kt] = th
                    else:
                        post_q.append(make_epack(e_tile, kt, th0_of[kt], th))
                inline_atp = (bi == BL - 1)
                post_q.append(make_mask_exp(e_tile, ex_row, ssum4, boxes,
                                            bi, inline_atp, nat_kc))
                post_q.append(make_softmax(ssum4, rinv,
                                           TT if inline_atp else EC))
                post_q.append(make_tail(bi, ex_row, rinv, boxes, nat_kc,
                                        inline_atp))
            while post_q:
                post_q.pop(0)()

    nc.compile()
    return nc


def _get_runner():
    if "runner" in _CACHE:
        return _CACHE["runner"]

    import jax
    from jax.sharding import Mesh, PartitionSpec
    from jax.experimental.shard_map import shard_map
    from concourse import bass2jax
    from concourse import mybir as _mb

    nc = _build_nc()
    bass2jax.install_neuronx_cc_hook()

    partition_name = (nc.partition_id_tensor.name
                      if nc.partition_id_tensor else None)
    in_names, out_names, out_avals, zero_outs = [], [], [], []
    for alloc in nc.m.functions[0].allocations:
        if not isinstance(alloc, _mb.MemoryLocationSet):
            continue
        name = alloc.memorylocations[0].name
        if alloc.kind == "ExternalInput":
            if name != partition_name:
                in_names.append(name)
        elif alloc.kind == "ExternalOutput":
            out_names.append(name)
            shape = tuple(alloc.tensor_shape)
            npdt = _mb.dt.np(alloc.dtype)
            out_avals.append(jax.core.ShapedArray(shape, npdt))
            zero_outs.append(np.zeros(shape, npdt))
    n_params = len(in_names)
    n_outs = len(out_names)
    all_in_names = in_names + out_names
    if partition_name is not None:
        all_in_names = all_in_names + [partition_name]
    donate = tuple(range(n_params, n_params + n_outs))

    def _body(*args):
        operands = list(args)
        if partition_name is not None:
            operands.append(bass2jax.partition_id_tensor())
        outs = bass2jax._bass_exec_p.bind(
            *operands,
            out_avals=tuple(out_avals),
            in_names=tuple(all_in_names),
            out_names=tuple(out_names),
            lowering_input_output_aliases=(),
            sim_require_finite=True,
            sim_require_nnan=True,
            nc=nc,
        )
        return tuple(outs)

    devices = jax.devices()[:NCORES]
    mesh = Mesh(np.asarray(devices), ("core",))
    in_specs = (PartitionSpec("core"),) * (n_params + n_outs)
    out_specs = (PartitionSpec("core"),) * n_outs
    sharded = jax.jit(
        shard_map(_body, mesh=mesh, in_specs=in_specs, out_specs=out_specs,
                  check_rep=False),
        donate_argnums=donate, keep_unused=True)

    def run(in_maps):
        concat_in = [
            np.concatenate([np.asarray(m[name]) for m in in_maps], axis=0)
            for name in in_names
        ]
        concat_zeros = [
            np.zeros((NCORES * z.shape[0], *z.shape[1:]), z.dtype)
            for z in zero_outs
        ]
        out_arrs = sharded(*concat_in, *concat_zeros)
        return [
            {name: np.asarray(out_arrs[i]).reshape(NCORES, *out_avals[i].shape)[c]
             for i, name in enumerate(out_names)}
            for c in range(NCORES)
        ]

    _CACHE["runner"] = run
    return run


def _make_in_maps(inputs):
    import ml_dtypes
    bf16 = ml_dtypes.bfloat16

    h_t = np.asarray(inputs["h_t"], dtype=np.float32)
    enc_out = np.asarray(inputs["enc_out"], dtype=np.float32)
    src_mask = np.asarray(inputs["src_mask"])
    Wa = np.asarray(inputs["Wa"], dtype=np.float32)
    Ua = np.asarray(inputs["Ua"], dtype=np.float32)
    va = np.asarray(inputs["va"], dtype=np.float32)

    # [KT, P, HT, P] column blocks of Ua.T / Wa.T (lhsT layouts)
    uaT = np.ascontiguousarray(
        Ua.T.reshape(HT, P, KT, P).transpose(2, 1, 0, 3)).astype(bf16)
    waT = np.ascontiguousarray(
        Wa.T.reshape(HT, P, KT, P).transpose(2, 1, 0, 3)).astype(bf16)
    va_pk = np.ascontiguousarray(va.reshape(KT, P).T).astype(bf16)   # [P, KT]
    encT = np.ascontiguousarray(
        enc_out.transpose(0, 2, 1).reshape(B, HT, P, TC, NT)
        .transpose(0, 3, 2, 1, 4)).astype(bf16)                 # [B, TC, P, HT, NT]
    encn = np.ascontiguousarray(
        enc_out.reshape(B, TT, P, TC, NT)
        .transpose(0, 3, 2, 1, 4)).astype(bf16)                 # [B, TC, P, TT, NT]
    mask_u8 = np.ascontiguousarray(src_mask.astype(np.uint8))

    in_maps = []
    for c in range(NCORES):
        sl = slice(c * BL, (c + 1) * BL)
        htT = np.ascontiguousarray(
            h_t[sl].T.reshape(HT, P, BL).transpose(1, 0, 2)
            .reshape(P, HT * BL)).astype(bf16)                  # [P, HT*BL]
        wall = np.empty((P, WX), dtype=bf16)
        wall[:, OFF_UA0:OFF_VA] = uaT[0].reshape(P, HT * P)
        wall[:, OFF_VA:OFF_HTT] = va_pk
        wall[:, OFF_HTT:OFF_WA0] = htT
        wall[:, OFF_WA0:OFF_REST] = waT[0].reshape(P, HT * P)
        for kt in range(1, KT):
            o = OFF_REST + (kt - 1) * 2048
            wall[:, o:o + 1024] = uaT[kt].reshape(P, HT * P)
            wall[:, o + 1024:o + 2048] = waT[kt].reshape(P, HT * P)
        in_maps.append({
            "encT": encT[sl],
            "encn": encn[sl],
            "wall": wall,
            "mask": mask_u8[sl],
        })
    return in_maps


def kernel(h_t, enc_out, src_mask, Wa, Ua, va):
    in_maps = _make_in_maps({
        "h_t": h_t, "enc_out": enc_out, "src_mask": src_mask,
        "Wa": Wa, "Ua": Ua, "va": va,
    })
    run = _get_runner()
    results = run(in_maps)
    context = np.concatenate([r["ctx"] for r in results], axis=0)
    attn = np.concatenate([r["attn"] for r in results], axis=0)
    return context, attn


# revision 35
# speedup vs baseline: 1.0302x; 1.0302x over previous
"""Bahdanau additive attention on 8 Trainium2 NeuronCores.

Data-parallel over batch: core c handles batches [4c, 4c+4).
Per batch b:
  ep[k,t]   = sum_h Ua[k,h] * enc[b,t,h]        (bf16 PE matmuls, k on PSUM partitions)
  z[k,t]    = tanh(ep[k,t] + hp[b,k])           (ScalarE, hp as per-partition bias)
  e[t]      = sum_k va[k] * z[k,t]              (M=1 bf16 PE matmuls, 4x col-packed)
  attn      = softmax(e) * mask renorm          (strip-exp out of PSUM; no max-sub)
  ctx[h]    = sum_t attn[t] * enc[b,t,h]        (M=1 bf16 PE matmuls, 4x col-packed,
                                                 1/sum folded into the exp-transpose)
hp[b,k] = sum_h Wa[k,h] * h_t[b,h] runs as per-kt N=4 matmul chains whose tiny
weight chunks trickle in with the staged weight DMAs, filling the DMA-starved
prologue. All one-time weights (Ua blocks, Wa blocks, va, h_t^T) are packed into
a single [128, WX] DRAM tensor loaded with a handful of staged DMAs (descriptor
issue on the Sync engine costs ~0.7us per dma_start).
"""

import numpy as np

import concourse.bass as bass
import concourse.tile as tile
from concourse import bacc, mybir

dt = mybir.dt
AF = mybir.ActivationFunctionType

B, T, H = 32, 1024, 1024
NCORES = 8
BL = B // NCORES          # batches per core
P = 128                   # partitions
NT = 512                  # matmul free-dim chunk (one PSUM bank of fp32)
KT = H // P               # k-tiles (output rows of ep)
HT = H // P               # h-tiles (contraction)
TT = T // P               # t-tiles (partition tiles of natural enc)
TC = T // NT              # t chunks per batch
EC = 4                    # e/ctx col-packed chunks
EW = T // EC              # 256

# packed-weight column offsets (bf16 elements per partition)
OFF_UA0 = 0
OFF_VA = 1024
OFF_HTT = 1032
OFF_WA0 = 1064
OFF_REST = 2088           # kt>=1: [uaT_kt (1024) | waT_kt (1024)] blocks
WX = OFF_REST + (KT - 1) * 2048


def _off_ua(kt):
    return OFF_UA0 if kt == 0 else OFF_REST + (kt - 1) * 2048


def _off_wa(kt):
    return OFF_WA0 if kt == 0 else OFF_REST + (kt - 1) * 2048 + 1024


_CACHE = {}


def _build_nc():
    nc = bacc.Bacc("TRN2", target_bir_lowering=False, debug=False)

    encT_d = nc.dram_tensor("encT", [BL, TC, P, HT, NT], dt.bfloat16,
                            kind="ExternalInput").ap()
    encn_d = nc.dram_tensor("encn", [BL, TC, P, TT, NT], dt.bfloat16,
                            kind="ExternalInput").ap()
    wall_d = nc.dram_tensor("wall", [P, WX], dt.bfloat16,
                            kind="ExternalInput").ap()
    mask_d = nc.dram_tensor("mask", [BL, T], dt.uint8, kind="ExternalInput").ap()

    ctx_d = nc.dram_tensor("ctx", [BL, H], dt.float32, kind="ExternalOutput").ap()
    attn_d = nc.dram_tensor("attn", [BL, T], dt.float32, kind="ExternalOutput").ap()

    with tile.TileContext(nc) as tc:
        from contextlib import ExitStack

        with ExitStack() as st:
            wpool = st.enter_context(tc.tile_pool(name="weights", bufs=1))
            etpool = st.enter_context(tc.tile_pool(name="encT", bufs=4))
            natpool = st.enter_context(tc.tile_pool(name="nat", bufs=4))
            thpool = st.enter_context(tc.tile_pool(name="tanh", bufs=4))
            smpool = st.enter_context(tc.tile_pool(name="small", bufs=1))
            pmain = st.enter_context(tc.tile_pool(name="pmain", bufs=4, space="PSUM"))
            pe_ps = st.enter_context(tc.tile_pool(name="pe", bufs=2, space="PSUM"))
            ptail = st.enter_context(tc.tile_pool(name="ptail", bufs=2, space="PSUM"))

            wall_sb = wpool.tile([P, WX], dt.bfloat16, tag="wall")

            def uaT_ap(kt, ht):
                o = _off_ua(kt) + ht * P
                return wall_sb[:, o:o + P]

            def waT_ap(kt, ht):
                o = _off_wa(kt) + ht * P
                return wall_sb[:, o:o + P]

            def htT_ap(ht):
                o = OFF_HTT + ht * BL
                return wall_sb[:, o:o + BL]

            def va_ap(kt):
                return wall_sb[:, OFF_VA + kt:OFF_VA + kt + 1]

            def load_wall(c0, c1):
                nc.sync.dma_start(wall_sb[:, c0:c1], wall_d[:, c0:c1])

            def load_encT(bi, tcc, eng, split=False):
                t_ = etpool.tile([P, HT, NT], dt.bfloat16, tag="encT",
                                 name=f"encT{bi}_{tcc}")
                if split:
                    # alternate halves across the two HWDGE queues so more
                    # DMA engines engage concurrently in the prologue
                    nc.sync.dma_start(t_[:, 0:2, :], encT_d[bi, tcc, :, 0:2, :])
                    nc.scalar.dma_start(t_[:, 2:4, :], encT_d[bi, tcc, :, 2:4, :])
                    nc.sync.dma_start(t_[:, 4:6, :], encT_d[bi, tcc, :, 4:6, :])
                    nc.scalar.dma_start(t_[:, 6:8, :], encT_d[bi, tcc, :, 6:8, :])
                else:
                    eng.dma_start(t_[:], encT_d[bi, tcc])
                return t_

            def load_nat(bi, kc):
                t_ = natpool.tile([P, TT, NT], dt.bfloat16, tag="nat",
                                  name=f"nat{bi}_{kc}")
                nc.scalar.dma_start(t_[:], encn_d[bi, kc])
                return t_

            def load_wall2(c0, c1, eng):
                eng.dma_start(wall_sb[:, c0:c1], wall_d[:, c0:c1])

            # prologue DMAs, need-ordered, striped across both queues:
            # per kt, Ua block rides the sync queue and Wa block the scalar
            # queue so both queues advance one half-block per group.
            load_wall2(OFF_UA0, OFF_UA0 + 512, nc.sync)       # uaT0 a
            load_wall2(OFF_UA0 + 512, OFF_VA, nc.scalar)      # uaT0 b
            load_wall2(OFF_VA, OFF_WA0, nc.sync)              # va + htT (tiny)
            encT_b0 = [load_encT(0, 0, None, split=True)]
            load_wall2(OFF_WA0, OFF_WA0 + 512, nc.sync)       # waT0 a
            load_wall2(OFF_WA0 + 512, OFF_REST, nc.scalar)    # waT0 b
            for _kt in range(1, KT):
                load_wall2(_off_ua(_kt), _off_wa(_kt), nc.sync)    # uaT_kt
                load_wall2(_off_wa(_kt), _off_ua(_kt + 1) if _kt < KT - 1
                           else WX, nc.scalar)                     # waT_kt

            # PE warm-up: back-to-back dummy matmuls while the prologue DMAs
            # stream in, so HAM reaches K=8/8 before the first real group.
            # The memset goes FIRST on the DVE queue so the warm-up isn't
            # delayed behind the 4.3us mask pass below.
            ones_b = wpool.tile([1, 1], dt.bfloat16, tag="ones_b")
            nc.vector.memset(ones_b[:], 1.0)
            hp_sb = wpool.tile([P, KT, BL], dt.float32, tag="hp")
            g_rhs = wpool.tile([P, NT], dt.bfloat16, tag="g_rhs")
            nc.vector.memset(g_rhs[:], 0.0)
            warm_ps = ptail.tile([P, NT], dt.float32, tag="tailps",
                                 name="warm_ps")
            for _ in range(16):
                nc.tensor.matmul(warm_ps[:], g_rhs[:, :P], g_rhs[:],
                                 start=True, stop=True)

            # mask for all batches: cast-DMA + one dual-op DVE pass (DVE is
            # idle in the prologue; ScalarE is not): mask_m1 = (mask-1)*1e30
            mask_f = smpool.tile([1, BL * T], dt.float32, tag="mask")
            nc.gpsimd.dma_start(mask_f[:], mask_d.rearrange("b t -> (b t)"))
            mask_m1 = smpool.tile([1, BL * T], dt.bfloat16, tag="mask_m1")
            nc.vector.tensor_scalar(mask_m1[:], mask_f[:], 1.0, 1e30,
                                    mybir.AluOpType.subtract,
                                    mybir.AluOpType.mult)
            encT_b0.append(load_encT(0, 1, None, split=True))

            # ---- deferred post-op FIFO: one item emitted per main group ----
            post_q = []

            def pop_post():
                if post_q:
                    post_q.pop(0)()

            def make_epack(e_tile, kt, th0, th1):
                def emit():
                    for c in range(EC):
                        th = th0 if c < TC else th1
                        nc.tensor.matmul(
                            e_tile[32 * c:32 * c + 1, :EW],
                            va_ap(kt),
                            th[:, (c % TC) * EW:(c % TC + 1) * EW],
                            start=(kt == 0), stop=False,
                            tile_position=(0, 32 * c))
                return emit

            def make_mask_exp(e_tile, ex_row, ssum4, boxes, bi, inline_atp,
                              nat_kc=None):
                def emit():
                    atp = ptail.tile([P, TT], dt.float32, tag="tailps",
                                     name=f"atp{bi}")
                    boxes["atp"] = atp
                    if inline_atp:
                        boxes["attnT"] = smpool.tile(
                            [P, TT], dt.bfloat16, tag="attnT", bufs=2,
                            name=f"attnT{bi}")
                        boxes["cp"] = ptail.tile([P, NT], dt.float32,
                                                 tag="tailps", name=f"cp{bi}")
                    for c in range(EC):
                        nc.tensor.matmul(
                            e_tile[32 * c:32 * c + 1, :EW],
                            ones_b[:],
                            mask_m1[0:1, bi * T + c * EW:bi * T + (c + 1) * EW],
                            start=False, stop=True,
                            tile_position=(0, 32 * c))
                    # strip-exp straight out of PSUM into a [1, T] row
                    # (partition shift 32c -> 0), then per-chunk sums on DVE
                    # pipelined behind the ScalarE exps. On the last batch the
                    # exps run in 8 half-strips and each half immediately
                    # feeds its exp-transpose + ctx matmuls (ScalarE/PE
                    # pipeline in the exposed tail).
                    if not inline_atp:
                        for c in range(EC):
                            nc.scalar.activation(
                                ex_row[:, c * EW:(c + 1) * EW],
                                e_tile[32 * c:32 * c + 1, :EW], AF.Exp)
                            nc.vector.tensor_reduce(
                                ssum4[:, c:c + 1],
                                ex_row[:, c * EW:(c + 1) * EW],
                                axis=mybir.AxisListType.X,
                                op=mybir.AluOpType.add)
                    else:
                        for tt in range(TT):
                            c, h2 = tt // 2, tt % 2
                            nc.scalar.activation(
                                ex_row[:, tt * P:(tt + 1) * P],
                                e_tile[32 * c:32 * c + 1,
                                       h2 * P:(h2 + 1) * P], AF.Exp)
                            nc.vector.tensor_reduce(
                                ssum4[:, tt:tt + 1],
                                ex_row[:, tt * P:(tt + 1) * P],
                                axis=mybir.AxisListType.X,
                                op=mybir.AluOpType.add)
                            nc.tensor.matmul(
                                atp[:, tt:tt + 1],
                                ex_row[:, tt * P:(tt + 1) * P],
                                ones_b[:], start=True, stop=True)
                            nc.vector.tensor_copy(
                                boxes["attnT"][:, tt:tt + 1],
                                atp[:, tt:tt + 1])
                            for cc in range(EC):
                                nc.tensor.matmul(
                                    boxes["cp"][32 * cc:32 * cc + 1, :EW],
                                    boxes["attnT"][:, tt:tt + 1],
                                    nat_kc[cc // TC][
                                        :, tt,
                                        (cc % TC) * EW:(cc % TC + 1) * EW],
                                    start=(tt == 0), stop=(tt == TT - 1),
                                    tile_position=(0, 32 * cc))
                return emit

            def make_softmax(ssum4, rinv, ncols):
                def emit():
                    ssum = smpool.tile([1, 1], dt.float32, tag="ssum", bufs=2)
                    nc.vector.tensor_reduce(ssum[:], ssum4[:, 0:ncols],
                                            axis=mybir.AxisListType.X,
                                            op=mybir.AluOpType.add)
                    nc.vector.reciprocal(rinv[:], ssum[:])
                return emit

            def make_tail(bi, ex_row, rinv, boxes, nat_kc, inline_atp):
                def emit():
                    # transpose UNnormalized exp into partitions: 1/sum is
                    # applied later on the ctx strips, so this does not wait
                    # for the softmax sum.
                    atp = boxes["atp"]
                    if not inline_atp:
                        for tt in range(TT):
                            nc.tensor.matmul(
                                atp[:, tt:tt + 1],
                                ex_row[:, tt * P:(tt + 1) * P],
                                ones_b[:], start=True, stop=True)
                        attnT = smpool.tile([P, TT], dt.bfloat16, tag="attnT",
                                            bufs=2)
                        nc.vector.tensor_copy(attnT[:], atp[:])
                    else:
                        attnT = boxes["attnT"]
                    # attn output: ex * (1/sum), full fp32 row
                    attn_sb = smpool.tile([1, T], dt.float32, tag="attn",
                                          bufs=2)
                    nc.vector.tensor_scalar_mul(attn_sb[:], ex_row[:], rinv[:])
                    nc.scalar.dma_start(attn_d[bi:bi + 1, :], attn_sb[:])
                    # context: normalize while draining the PSUM strips
                    if not inline_atp:
                        cp = ptail.tile([P, NT], dt.float32, tag="tailps",
                                        name=f"cp{bi}")
                        for tt in range(TT):
                            for c in range(EC):
                                nc.tensor.matmul(
                                    cp[32 * c:32 * c + 1, :EW],
                                    attnT[:, tt:tt + 1],
                                    nat_kc[c // TC][
                                        :, tt,
                                        (c % TC) * EW:(c % TC + 1) * EW],
                                    start=(tt == 0), stop=(tt == TT - 1),
                                    tile_position=(0, 32 * c))
                    else:
                        cp = boxes["cp"]
                    ctx_sb = smpool.tile([1, H], dt.float32, tag="ctx", bufs=2)
                    for c in range(EC):
                        if c % 2 == 0:
                            nc.vector.tensor_scalar_mul(
                                ctx_sb[:, c * EW:(c + 1) * EW],
                                cp[32 * c:32 * c + 1, :EW], rinv[:])
                        else:
                            nc.scalar.mul(
                                ctx_sb[:, c * EW:(c + 1) * EW],
                                cp[32 * c:32 * c + 1, :EW], rinv[:])
                    nc.scalar.dma_start(ctx_d[bi:bi + 1, :], ctx_sb[:])
                return emit

            # ---- main loop: tcc-outer for batch 0 (DMA need-order),
            # kt-outer for the rest (both encT tiles prefetched) ----
            for bi in range(BL):
                if bi == 0:
                    encT_t = encT_b0
                    group_iter = [(kt, tcc) for tcc in range(TC)
                                  for kt in range(KT)]
                elif bi == 1:
                    encT_t = encT_b1
                else:
                    encT_t = encT_next
                if bi > 0:
                    group_iter = [(kt, tcc) for kt in range(KT)
                                  for tcc in range(TC)]
                e_tile = pe_ps.tile([P, NT], dt.float32, tag="e",
                                    name=f"e_ps{bi}")
                ex_row = smpool.tile([1, T], dt.bfloat16, tag="ex", bufs=2,
                                     name=f"ex{bi}")
                ssum4 = smpool.tile([1, TT], dt.float32, tag="ssum4", bufs=2,
                                    name=f"ssum4_{bi}")
                rinv = smpool.tile([1, 1], dt.float32, tag="rinv", bufs=2,
                                   name=f"rinv{bi}")
                boxes = {}
                th0_of = {}
                for gi, (kt, tcc) in enumerate(group_iter):
                    # prefetch emission points
                    if bi == 0:
                        if gi == 10:
                            encT_b1 = [load_encT(1, 0, nc.sync)]
                            nat_kc = [load_nat(bi, 0)]
                        elif gi == 12:
                            encT_b1.append(load_encT(1, 1, nc.sync))
                            nat_kc.append(load_nat(bi, 1))
                    else:
                        if gi == 2 and bi < BL - 1:
                            encT_next = [load_encT(bi + 1, 0, nc.sync)]
                        elif gi == 6 and bi < BL - 1:
                            encT_next.append(load_encT(bi + 1, 1, nc.sync))
                        if gi == 10:
                            nat_kc = [load_nat(bi, 0)]
                        elif gi == 12:
                            nat_kc.append(load_nat(bi, 1))
                    ps = pmain.tile([P, NT], dt.float32, tag="big")
                    for ht in range(HT):
                        nc.tensor.matmul(
                            ps[:], uaT_ap(kt, ht), encT_t[tcc][:, ht, :],
                            start=(ht == 0), stop=(ht == HT - 1))
                    if bi == 0 and tcc == 0:
                        hp_ps = ptail.tile([P, BL], dt.float32, tag="tailps",
                                           name=f"hp_ps{kt}")
                        for ht in range(HT):
                            nc.tensor.matmul(
                                hp_ps[:], waT_ap(kt, ht), htT_ap(ht),
                                start=(ht == 0), stop=(ht == HT - 1))
                        nc.vector.tensor_copy(hp_sb[:, kt, :], hp_ps[:])
                    th = thpool.tile([P, NT], dt.bfloat16, tag="th",
                                     bufs=12, name="th")
                    nc.scalar.activation(th[:], ps[:], AF.Tanh,
                                         bias=hp_sb[:, kt, bi:bi + 1])
                    pop_post()
                    if tcc == 0:
                        th0_of[kt] = th
                    else:
                        post_q.append(make_epack(e_tile, kt, th0_of[kt], th))
                inline_atp = (bi == BL - 1)
                post_q.append(make_mask_exp(e_tile, ex_row, ssum4, boxes,
                                            bi, inline_atp, nat_kc))
                post_q.append(make_softmax(ssum4, rinv,
                                           TT if inline_atp else EC))
                post_q.append(make_tail(bi, ex_row, rinv, boxes, nat_kc,
                                        inline_atp))
            while post_q:
                post_q.pop(0)()

    nc.compile()
    return nc


def _get_runner():
    if "runner" in _CACHE:
        return _CACHE["runner"]

    import jax
    from jax.sharding import Mesh, PartitionSpec
    from jax.experimental.shard_map import shard_map
    from concourse import bass2jax
    from concourse import mybir as _mb

    nc = _build_nc()
    bass2jax.install_neuronx_cc_hook()

    partition_name = (nc.partition_id_tensor.name
                      if nc.partition_id_tensor else None)
    in_names, out_names, out_avals, zero_outs = [], [], [], []
    for alloc in nc.m.functions[0].allocations:
        if not isinstance(alloc, _mb.MemoryLocationSet):
            continue
        name = alloc.memorylocations[0].name
        if alloc.kind == "ExternalInput":
            if name != partition_name:
                in_names.append(name)
        elif alloc.kind == "ExternalOutput":
            out_names.append(name)
            shape = tuple(alloc.tensor_shape)
            npdt = _mb.dt.np(alloc.dtype)
            out_avals.append(jax.core.ShapedArray(shape, npdt))
            zero_outs.append(np.zeros(shape, npdt))
    n_params = len(in_names)
    n_outs = len(out_names)
    all_in_names = in_names + out_names
    if partition_name is not None:
        all_in_names = all_in_names + [partition_name]
    donate = tuple(range(n_params, n_params + n_outs))

    def _body(*args):
        operands = list(args)
        if partition_name is not None:
            operands.append(bass2jax.partition_id_tensor())
        outs = bass2jax._bass_exec_p.bind(
            *operands,
            out_avals=tuple(out_avals),
            in_names=tuple(all_in_names),
            out_names=tuple(out_names),
            lowering_input_output_aliases=(),
            sim_require_finite=True,
            sim_require_nnan=True,
            nc=nc,
        )
        return tuple(outs)

    devices = jax.devices()[:NCORES]
    mesh = Mesh(np.asarray(devices), ("core",))
    in_specs = (PartitionSpec("core"),) * (n_params + n_outs)
    out_specs = (PartitionSpec("core"),) * n_outs
    sharded = jax.jit(
        shard_map(_body, mesh=mesh, in_specs=in_specs, out_specs=out_specs,
                  check_rep=False),
        donate_argnums=donate, keep_unused=True)

    def run(in_maps):
        concat_in = [
            np.concatenate([np.asarray(m[name]) for m in in_maps], axis=0)
            for name in in_names
        ]
        concat_zeros = [
            np.zeros((NCORES * z.shape[0], *z.shape[1:]), z.dtype)
            for z in zero_outs
        ]
        out_arrs = sharded(*concat_in, *concat_zeros)
        return [
            {name: np.asarray(out_arrs[i]).reshape(NCORES, *out_avals[i].shape)[c]
             for i, name in enumerate(out_names)}
            for c in range(NCORES)
        ]

    _CACHE["runner"] = run
    return run


def _make_in_maps(inputs):
    import ml_dtypes
    bf16 = ml_dtypes.bfloat16

    h_t = np.asarray(inputs["h_t"], dtype=np.float32)
    enc_out = np.asarray(inputs["enc_out"], dtype=np.float32)
    src_mask = np.asarray(inputs["src_mask"])
    Wa = np.asarray(inputs["Wa"], dtype=np.float32)
    Ua = np.asarray(inputs["Ua"], dtype=np.float32)
    va = np.asarray(inputs["va"], dtype=np.float32)

    # [KT, P, HT, P] column blocks of Ua.T / Wa.T (lhsT layouts)
    uaT = np.ascontiguousarray(
        Ua.T.reshape(HT, P, KT, P).transpose(2, 1, 0, 3)).astype(bf16)
    waT = np.ascontiguousarray(
        Wa.T.reshape(HT, P, KT, P).transpose(2, 1, 0, 3)).astype(bf16)
    va_pk = np.ascontiguousarray(va.reshape(KT, P).T).astype(bf16)   # [P, KT]
    encT = np.ascontiguousarray(
        enc_out.transpose(0, 2, 1).reshape(B, HT, P, TC, NT)
        .transpose(0, 3, 2, 1, 4)).astype(bf16)                 # [B, TC, P, HT, NT]
    encn = np.ascontiguousarray(
        enc_out.reshape(B, TT, P, TC, NT)
        .transpose(0, 3, 2, 1, 4)).astype(bf16)                 # [B, TC, P, TT, NT]
    mask_u8 = np.ascontiguousarray(src_mask.astype(np.uint8))

    in_maps = []
    for c in range(NCORES):
        sl = slice(c * BL, (c + 1) * BL)
        htT = np.ascontiguousarray(
            h_t[sl].T.reshape(HT, P, BL).transpose(1, 0, 2)
            .reshape(P, HT * BL)).astype(bf16)                  # [P, HT*BL]
        wall = np.empty((P, WX), dtype=bf16)
        wall[:, OFF_UA0:OFF_VA] = uaT[0].reshape(P, HT * P)
        wall[:, OFF_VA:OFF_HTT] = va_pk
        wall[:, OFF_HTT:OFF_WA0] = htT
        wall[:, OFF_WA0:OFF_REST] = waT[0].reshape(P, HT * P)
        for kt in range(1, KT):
            o = OFF_REST + (kt - 1) * 2048
            wall[:, o:o + 1024] = uaT[kt].reshape(P, HT * P)
            wall[:, o + 1024:o + 2048] = waT[kt].reshape(P, HT * P)
        in_maps.append({
            "encT": encT[sl],
            "encn": encn[sl],
            "wall": wall,
            "mask": mask_u8[sl],
        })
    return in_maps


def kernel(h_t, enc_out, src_mask, Wa, Ua, va):
    in_maps = _make_in_maps({
        "h_t": h_t, "enc_out": enc_out, "src_mask": src_mask,
        "Wa": Wa, "Ua": Ua, "va": va,
    })
    run = _get_runner()
    results = run(in_maps)
    context = np.concatenate([r["ctx"] for r in results], axis=0)
    attn = np.concatenate([r["attn"] for r in results], axis=0)
    return context, attn
